# revision 6
# baseline (speedup 1.0000x reference)
"""Trainium2 Bass kernel for the CAM (channel-attention) block.

Reference math (per batch b):
    A    = inputs[b].reshape(HW, C)                      # [4096, 512]
    G    = A^T @ A                                       # [C, C] gram
    attn = softmax(G, axis=-1)
    out  = gamma * (A @ attn^T) + A                      # [HW, C]

Sharding: data-parallel over batch. 16 batches / 8 cores = 2 per core.
Each core runs the same NEFF on its own batch slice; no collectives.

Per-core kernel structure (all fp32):
  1. Load A (both batches, double-buffered) into SBUF.
  2. Gram via PE: for each 128-wide c-chunk, accumulate 32 matmuls
     (contraction over n) into a PSUM bank; copy to SBUF (G tiles).
     By symmetry G == G^T, so the same SBUF tiles serve as the
     column-layout [d, c] operand later.
  3. Softmax without any 512x512 transpose:
       m_c   = row max (DVE free-axis reduce)
       m_row = [1, 512] via 4 tiny PE transposes of the [128,1] maxes
       mb    = broadcast of m_row across partitions (K=1 ones matmul)
       E     = exp(G - mb)          (DVE sub + ACT exp)  == exp in [d,c] layout
       Z     = ones^T @ E           (K=128 ones matmul -> [1, 512])
       zb    = broadcast of 1/Z     (K=1 ones matmul)
       attnT = E * zb               (DVE)  == softmax(G)^T in [d, c] layout
  4. Second matmul contracts over d, so A^T tiles are produced on the fly
     with PE transposes (4 per 128-row n-tile, packed into one PSUM bank),
     copied to SBUF, then 4 matmuls accumulate A @ attn^T for that n-tile.
  5. Fused epilogue on DVE: out = (psum * gamma) + A  (scalar_tensor_tensor),
     then DMA the [128, 512] tile to HBM.
"""

import numpy as np

import concourse.bass as bass
import concourse.mybir as mybir
import concourse.tile as tile
from concourse import bacc
from concourse.bass_utils import run_bass_kernel_spmd
from concourse.masks import make_identity

B, H, W, C = 16, 64, 64, 512
N = H * W  # 4096
NCORES = 8
BPC = B // NCORES  # batches per core
NT = N // 128  # 32 row tiles per batch
CT = C // 128  # 4 channel chunks

F32 = mybir.dt.float32


def _build_bass(reps: int = 1) -> bass.Bass:
    nc = bacc.Bacc("TRN2", target_bir_lowering=False, debug=False, num_devices=NCORES)

    x = nc.dram_tensor("x", [BPC, N, C], F32, kind="ExternalInput").ap()
    gamma = nc.dram_tensor("gamma", [1], F32, kind="ExternalInput").ap()
    out = nc.dram_tensor("out", [BPC, N, C], F32, kind="ExternalOutput").ap()

    with tile.TileContext(nc) as tc:
        for _ in range(reps):
            _emit(tc, out, x, gamma)
    nc.compile()
    return nc


def _emit(tc: tile.TileContext, out: bass.AP, x: bass.AP, gamma: bass.AP):
    nc = tc.nc
    mult = mybir.AluOpType.mult
    add = mybir.AluOpType.add

    # [b, p, i, d] view: row n = i*128 + p
    x_r = x.rearrange("b (i p) d -> b p i d", p=128)
    out_r = out.rearrange("b (i p) d -> b p i d", p=128)

    with (
        tc.tile_pool(name="abig", bufs=2) as pa,
        tc.tile_pool(name="smx", bufs=2) as psx,
        tc.tile_pool(name="single", bufs=1) as pone,
        tc.tile_pool(name="small", bufs=2) as psm,
        tc.tile_pool(name="work", bufs=3) as pw,
        tc.tile_pool(name="pgram", bufs=2, space="PSUM") as pg,
        tc.tile_pool(name="psmx", bufs=1, space="PSUM") as pps,
        tc.tile_pool(name="ptr", bufs=3, space="PSUM") as ptr,
        tc.tile_pool(name="pout", bufs=2, space="PSUM") as pot,
    ):
        ident = pone.tile([128, 128], F32)
        make_identity(nc, ident)
        ones_k = pone.tile([128, 1], F32)
        nc.vector.memset(ones_k, 1.0)
        ones_r = pone.tile([1, 128], F32)
        nc.vector.memset(ones_r, 1.0)
        gamma_sb = pone.tile([128, 1], F32)
        nc.sync.dma_start(out=gamma_sb, in_=gamma.to_broadcast([128, 1]))

        for b in range(BPC):
            A = pa.tile([128, NT, C], F32, tag="A")
            nc.sync.dma_start(out=A, in_=x_r[b])

            # ---- gram: G[c-chunk] = A^T A rows, [128, 512] each ----
            G = []
            m = []
            for c in range(CT):
                gps = pg.tile([128, C], F32, tag="g")
                for k in range(NT):
                    nc.tensor.matmul(
                        gps,
                        lhsT=A[:, k, c * 128 : (c + 1) * 128],
                        rhs=A[:, k, :],
                        start=(k == 0),
                        stop=(k == NT - 1),
                    )
                g_sb = psx.tile([128, C], F32, tag=f"G{c}", name=f"g_sb{c}")
                nc.scalar.copy(g_sb, gps)
                mc = psm.tile([128, 1], F32, tag=f"m{c}", name=f"m{c}")
                nc.vector.reduce_max(mc, gps, axis=mybir.AxisListType.X)
                G.append(g_sb)
                m.append(mc)

            # ---- row maxes -> [1, 512] row, broadcast across partitions ----
            mrow_ps = pps.tile([1, C], F32, tag="s", name="mrow_ps")
            for c in range(CT):
                nc.tensor.matmul(
                    mrow_ps[:, c * 128 : (c + 1) * 128],
                    lhsT=m[c],
                    rhs=ident,
                    start=True,
                    stop=True,
                )
            mrow = psm.tile([1, C], F32, tag="mrow", name="mrow")
            nc.vector.tensor_copy(mrow, mrow_ps)
            mb_ps = pps.tile([128, C], F32, tag="s", name="mb_ps")
            nc.tensor.matmul(mb_ps, lhsT=ones_r, rhs=mrow, start=True, stop=True)

            # ---- E = exp(G - m_bcast) in [d, c] layout (G is symmetric) ----
            E = []
            for t in range(CT):
                nc.vector.tensor_sub(G[t], G[t], mb_ps)
                e_sb = psx.tile([128, C], F32, tag=f"E{t}", name=f"e_sb{t}")
                nc.scalar.activation(e_sb, G[t], mybir.ActivationFunctionType.Exp)
                E.append(e_sb)

            # ---- Z = column sums via ones matmul; zb = broadcast(1/Z) ----
            z_ps = pps.tile([1, C], F32, tag="s", name="z_ps")
            for t in range(CT):
                nc.tensor.matmul(
                    z_ps, lhsT=ones_k, rhs=E[t], start=(t == 0), stop=(t == CT - 1)
                )
            zr = psm.tile([1, C], F32, tag="zr", name="zr")
            nc.vector.reciprocal(zr, z_ps)
            zb_ps = pps.tile([128, C], F32, tag="s", name="zb_ps")
            nc.tensor.matmul(zb_ps, lhsT=ones_r, rhs=zr, start=True, stop=True)
            for t in range(CT):
                nc.vector.tensor_mul(E[t], E[t], zb_ps)  # E := attn^T tiles

            # ---- out rows: transpose A blocks, matmul, fused epilogue ----
            for i in range(NT):
                trp = ptr.tile([128, C], F32, tag="tr", name="trp")
                for t in range(CT):
                    nc.tensor.matmul(
                        trp[:, t * 128 : (t + 1) * 128],
                        lhsT=A[:, i, t * 128 : (t + 1) * 128],
                        rhs=ident,
                        start=True,
                        stop=True,
                    )
                at = pw.tile([128, C], F32, tag="at", name="at")
                nc.any.tensor_copy(at, trp)
                ops = pot.tile([128, C], F32, tag="o", name="ops")
                for t in range(CT):
                    nc.tensor.matmul(
                        ops,
                        lhsT=at[:, t * 128 : (t + 1) * 128],
                        rhs=E[t],
                        start=(t == 0),
                        stop=(t == CT - 1),
                    )
                ot = pw.tile([128, C], F32, tag="ot", name="ot")
                nc.vector.scalar_tensor_tensor(
                    out=ot, in0=ops, scalar=gamma_sb, in1=A[:, i, :], op0=mult, op1=add
                )
                nc.sync.dma_start(out=out_r[b][:, i, :], in_=ot)


_NC_CACHE = None


def _get_nc():
    global _NC_CACHE
    if _NC_CACHE is None:
        _NC_CACHE = _build_bass()
    return _NC_CACHE


def kernel(**inputs) -> np.ndarray:
    x = np.ascontiguousarray(np.asarray(inputs["inputs"], dtype=np.float32)).reshape(
        B, N, C
    )
    gamma = np.ascontiguousarray(np.asarray(inputs["gamma"], dtype=np.float32))

    nc = _get_nc()
    in_maps = [
        {"x": np.ascontiguousarray(x[i * BPC : (i + 1) * BPC]), "gamma": gamma}
        for i in range(NCORES)
    ]
    res = run_bass_kernel_spmd(nc, in_maps, core_ids=list(range(NCORES)))
    outs = [res.results[i]["out"] for i in range(NCORES)]
    return np.concatenate(outs, axis=0).reshape(B, H, W, C)


# revision 32
# speedup vs baseline: 589.9710x; 589.9710x over previous
"""Trainium2 Bass kernel for the CAM (channel-attention) block.

Reference math (per batch b):
    A    = inputs[b].reshape(HW, C)                      # [4096, 512]
    G    = A^T @ A                                       # [C, C] gram
    attn = softmax(G, axis=-1)
    out  = gamma * (A @ attn^T) + A                      # [HW, C]

Data-parallel over batch: 16 batches / 8 cores = 2 per core, same NEFF.

Per-core schedule (emission order == engine static order):
  - Chunked loads for both batches on the SP HWDGE ring.
  - Gram in fp32r (single-pass PE mode), k-major over load chunks, all four
    row-chunks accumulating in four PSUM banks; G is symmetric so only the
    upper blocks are computed (row widths 512/384/256/256) and the missing
    lower blocks are mirrored with five small PE transposes.
  - Softmax without any 512x512 transpose: row maxes (DVE) are transposed
    into a [1,512] row (PE), broadcast via K=1 ones-matmuls, E = exp(G-m)
    (ACT, fp32r out), Z = ones^T E (K=128 matmul), attnT = E * broadcast(1/Z)
    cast to bf16 (DVE).
  - Second matmul contracts over d, so A^T blocks are made on the fly:
    bf16 cast (GPSIMD) -> 4 PE transposes -> ACT copy to SBUF, then 4
    matmuls accumulate A @ attn^T per 128-row tile; epilogue fuses
    out = gamma*psum + A in one DVE op; store on SP ring.
  - The transpose stages for batch 0 are interleaved into batch 0's gram
    (filling DMA-paced gaps), batch 0's matmul groups into batch 1's gram
    window, and the remainder runs as a 2-deep software pipeline.
"""

import numpy as np

import concourse.bass as bass
import concourse.mybir as mybir
import concourse.tile as tile
from concourse import bacc
from concourse.bass_utils import run_bass_kernel_spmd
from concourse.masks import make_identity

B, H, W, C = 16, 64, 64, 512
N = H * W  # 4096
NCORES = 8
BPC = B // NCORES  # batches per core
NT = N // 128  # 32 row tiles per batch
CT = C // 128  # 4 channel chunks

F32 = mybir.dt.float32
F32R = mybir.dt.float32r  # single-pass PE mode: full rate at free-dim >= 256
BF16 = mybir.dt.bfloat16


def _build_bass(reps: int = 1) -> bass.Bass:
    nc = bacc.Bacc("TRN2", target_bir_lowering=False, debug=False, num_devices=NCORES)

    x = nc.dram_tensor("x", [BPC, N, C], F32, kind="ExternalInput").ap()
    gamma = nc.dram_tensor("gamma", [1], F32, kind="ExternalInput").ap()
    out = nc.dram_tensor("out", [BPC, N, C], F32, kind="ExternalOutput").ap()

    with tile.TileContext(nc) as tc:
        for _ in range(reps):
            _emit(tc, out, x, gamma)
    nc.compile()
    return nc


def _emit(tc: tile.TileContext, out: bass.AP, x: bass.AP, gamma: bass.AP):
    nc = tc.nc
    mult = mybir.AluOpType.mult
    add = mybir.AluOpType.add

    # [b, p, i, d] view: row n = i*128 + p
    x_r = x.rearrange("b (i p) d -> b p i d", p=128)
    out_r = out.rearrange("b (i p) d -> b p i d", p=128)

    KC = 16  # A-load chunks per batch; gram starts once chunk 0 lands
    KCS = NT // KC

    with (
        tc.tile_pool(name="abig", bufs=2) as pa,
        tc.tile_pool(name="smx", bufs=2) as psx,
        tc.tile_pool(name="single", bufs=1) as pone,
        tc.tile_pool(name="small", bufs=2) as psm,
        tc.tile_pool(name="work", bufs=3) as pw,
        tc.tile_pool(name="pgram", bufs=1, space="PSUM") as pg,
        tc.tile_pool(name="psmx", bufs=1, space="PSUM") as pps,
        tc.tile_pool(name="ptrm", bufs=3, space="PSUM") as ptm,
    ):
        ident = pone.tile([128, 128], F32)
        make_identity(nc, ident)
        ident_bf = pone.tile([128, 128], BF16)
        nc.vector.tensor_copy(ident_bf, ident)
        ones_k = pone.tile([128, 1], F32)
        nc.vector.memset(ones_k, 1.0)
        ones_kr = pone.tile([128, 1], F32)
        nc.vector.tensor_copy(ones_kr.bitcast(F32R), ones_k)
        ones_r = pone.tile([1, 128], F32)
        nc.vector.memset(ones_r, 1.0)
        gamma_sb = pone.tile([128, 1], F32)
        nc.sync.dma_start(out=gamma_sb, in_=gamma.to_broadcast([128, 1]))

        # ---- stage all batch loads first (SP HWDGE ring, chunked) ----
        As = []
        for b in range(BPC):
            A = pa.tile([128, NT, C], F32, tag="A", name=f"A{b}")
            # single-tile leading chunks let the first gram matmuls start
            # ~1.5us earlier; the rest go in 2-tile chunks
            bounds = [0, 1, 2] + list(range(4, NT + 1, 2)) if b == 0 else list(
                range(0, NT + 1, KCS)
            )
            for lo_, hi_ in zip(bounds[:-1], bounds[1:]):
                nc.sync.dma_start(
                    out=A[:, lo_:hi_, :].bitcast(F32R),
                    in_=x_r[b][:, lo_:hi_, :].bitcast(F32R),
                )
            As.append(A)

        # ---- mm2 stage helpers -------------------------------------------
        steps = [(b, i) for b in range(BPC) for i in range(NT)]
        at_q = {}
        Ebs = []

        def stage1(idx):
            """bf16 cast (GPSIMD) -> PE transposes -> ACT copy to SBUF."""
            b, i = steps[idx]
            abf = pw.tile([128, C], BF16, tag="abf", name="abf", bufs=6)
            nc.gpsimd.tensor_copy(abf, As[b][:, i, :])
            trp = ptm.tile([128, C], BF16, tag="trm", name="trp")
            for t in range(CT):
                nc.tensor.transpose(
                    trp[:, t * 128 : (t + 1) * 128],
                    abf[:, t * 128 : (t + 1) * 128],
                    ident_bf,
                )
            at = pw.tile([128, C], BF16, tag="at", name="at", bufs=28)
            nc.scalar.copy(at, trp)
            at_q[idx] = at

        def mmgroup(idx):
            """4 accumulating matmuls + fused epilogue + store."""
            b, j = steps[idx]
            at = at_q.pop(idx)
            if idx >= NT:
                # tail phase: gram banks are free again; rotating over them
                # deepens the ops pipeline beyond the 3 shared trm slots
                ops = pg.tile([128, C], F32, tag=f"g{idx % CT}", name="ops", bufs=1)
            else:
                ops = ptm.tile([128, C], F32, tag="trm", name="ops")
            for t in range(CT):
                nc.tensor.matmul(
                    ops,
                    lhsT=at[:, t * 128 : (t + 1) * 128],
                    rhs=Ebs[b][t],
                    start=(t == 0),
                    stop=(t == CT - 1),
                )
            ot = pw.tile([128, C], F32, tag="ot", name="ot", bufs=6)
            nc.vector.scalar_tensor_tensor(
                out=ot, in0=ops, scalar=gamma_sb, in1=As[b][:, j, :], op0=mult, op1=add
            )
            nc.sync.dma_start(out=out_r[b][:, j, :], in_=ot)

        staged = 0  # next step to stage
        consumed = 0  # next step to run mm for
        n_steps = len(steps)

        def fill_stage(n):
            nonlocal staged
            for _ in range(n):
                if staged < n_steps and staged - consumed < 28:
                    stage1(staged)
                    staged += 1

        def fill_mm(n, limit):
            nonlocal consumed
            for _ in range(n):
                if consumed < min(staged, limit):
                    mmgroup(consumed)
                    consumed += 1

        # ---- gram + softmax per batch ----
        lo = [0, 128, 256, 256]  # computed free-range start per row (symmetry)
        for b in range(BPC):
            A = As[b]
            Ar = A.bitcast(F32R)
            # mm-groups for batch b need Eb[b]; only earlier batches' groups
            # may be emitted inside this batch's gram/softmax section.
            mm_limit = b * NT

            # gram: k-major over load chunks, 4 PSUM banks; fill DMA-paced
            # gaps with transpose staging (b=0) / batch-0 mm groups (b=1).
            gps = [
                pg.tile([128, C], F32, tag=f"g{c}", name=f"gps{b}_{c}", bufs=1)
                for c in range(CT)
            ]
            for kc in range(KC):
                for c in range(CT):
                    for k in range(kc * KCS, (kc + 1) * KCS):
                        nc.tensor.matmul(
                            gps[c][:, lo[c] :],
                            lhsT=Ar[:, k, c * 128 : (c + 1) * 128],
                            rhs=Ar[:, k, lo[c] :],
                            start=(k == 0),
                            stop=(k == NT - 1),
                        )
                if b == 0:
                    if kc >= 2:
                        fill_stage(2)
                else:
                    if kc >= 2:
                        fill_mm(1, mm_limit)
                        fill_stage(1)

            G = [None] * CT
            for c in range(CT):
                g_sb = psx.tile([128, C], F32, tag=f"G{c}", name=f"g_sb{c}", bufs=1)
                nc.scalar.copy(g_sb[:, lo[c] :], gps[c][:, lo[c] :])
                G[c] = g_sb

            # fillers before each PE wait-point of the softmax chain keep the
            # in-order PE stream fed while DVE/ACT latency drains.
            def filler(n):
                if b == 0:
                    fill_stage(n)
                else:
                    fill_mm(n, mm_limit)
                    fill_stage(1)

            filler(2)
            # mirror lower blocks: G[c][:, s*128:..] = T(G[s][:, c*128:..])
            mir = [(1, 0), (2, 0), (2, 1), (3, 0), (3, 1)]
            mir_ps = ptm.tile([128, C], F32, tag="trm", name="mir_ps")
            mir2_ps = pps.tile([128, 128], F32, tag="s", name="mir2_ps")
            for n_, (c, s) in enumerate(mir):
                dst = mir_ps[:, n_ * 128 : (n_ + 1) * 128] if n_ < 4 else mir2_ps
                nc.tensor.transpose(dst, G[s][:, c * 128 : (c + 1) * 128], ident)
            for n_, (c, s) in enumerate(mir):
                srcp = mir_ps[:, n_ * 128 : (n_ + 1) * 128] if n_ < 4 else mir2_ps
                nc.scalar.copy(G[c][:, s * 128 : (s + 1) * 128], srcp)

            # row maxes -> [1, 512] row -> broadcast across partitions
            m = []
            for c in range(CT):
                mc = psm.tile([128, 1], F32, tag=f"m{c}", name=f"m{c}")
                nc.vector.reduce_max(mc, G[c], axis=mybir.AxisListType.X)
                m.append(mc)
            filler(2)
            mrow_ps = pps.tile([1, C], F32, tag="s", name="mrow_ps")
            for c in range(CT):
                nc.tensor.transpose(mrow_ps[:, c * 128 : (c + 1) * 128], m[c], ident)
            mrow = psm.tile([1, C], F32, tag="mrow", name="mrow")
            nc.vector.tensor_copy(mrow, mrow_ps)
            filler(1)
            mb_ps = pps.tile([128, C], F32, tag="s", name="mb_ps")
            nc.tensor.matmul(mb_ps, lhsT=ones_r, rhs=mrow, start=True, stop=True)

            # E = exp(G - m_bcast) in [d, c] layout (G is symmetric)
            E = []
            for t in range(CT):
                nc.vector.tensor_sub(G[t], G[t], mb_ps)
                e_sb = psx.tile([128, C], F32, tag=f"E{t}", name=f"e_sb{t}", bufs=1)
                nc.scalar.activation(
                    e_sb.bitcast(F32R), G[t], mybir.ActivationFunctionType.Exp
                )
                E.append(e_sb)

            filler(3)
            # Z = column sums via ones matmul; zb = broadcast(1/Z)
            z_ps = pps.tile([1, C], F32, tag="s", name="z_ps")
            for t in range(CT):
                nc.tensor.matmul(
                    z_ps,
                    lhsT=ones_kr.bitcast(F32R),
                    rhs=E[t].bitcast(F32R),
                    start=(t == 0),
                    stop=(t == CT - 1),
                )
            zr = psm.tile([1, C], F32, tag="zr", name="zr")
            nc.vector.reciprocal(zr, z_ps)
            filler(1)
            zb_ps = pps.tile([128, C], F32, tag="s", name="zb_ps")
            nc.tensor.matmul(zb_ps, lhsT=ones_r, rhs=zr, start=True, stop=True)
            Eb = []
            for t in range(CT):
                e_bf = psx.tile([128, C], BF16, tag=f"Eb{t}", name=f"e_bf{t}")
                nc.vector.tensor_mul(e_bf, E[t], zb_ps)  # attn^T, bf16
                Eb.append(e_bf)
            Ebs.append(Eb)

        # ---- remaining mm2 steps: 4-deep software pipeline ----
        while consumed < n_steps:
            if staged < n_steps and staged - consumed < 4:
                stage1(staged)
                staged += 1
            else:
                mmgroup(consumed)
                consumed += 1


_NC_CACHE = None


def _get_nc():
    global _NC_CACHE
    if _NC_CACHE is None:
        _NC_CACHE = _build_bass()
    return _NC_CACHE


def kernel(**inputs) -> np.ndarray:
    x = np.ascontiguousarray(np.asarray(inputs["inputs"], dtype=np.float32)).reshape(
        B, N, C
    )
    gamma = np.ascontiguousarray(np.asarray(inputs["gamma"], dtype=np.float32))

    nc = _get_nc()
    in_maps = [
        {"x": np.ascontiguousarray(x[i * BPC : (i + 1) * BPC]), "gamma": gamma}
        for i in range(NCORES)
    ]
    res = run_bass_kernel_spmd(nc, in_maps, core_ids=list(range(NCORES)))
    outs = [res.results[i]["out"] for i in range(NCORES)]
    return np.concatenate(outs, axis=0).reshape(B, H, W, C)
